# revision 2
# baseline (speedup 1.0000x reference)
"""LSTM decoder (constant input per step, ragged lengths) on 8 TRN2 cores.

Per-core math (16 sequences, H=512, T=511):
    gates_t = x_proj + h_t @ W_hh.T ; standard LSTM cell; ys[b,t] = h_{t+1}

Structure (vs. a 3-pass fp32r baseline, ~1.3x faster at equal conditions):
  * 2-pass exact product: the stationary packs [h_hi | pad | h_lo] (48 of
    128 PE cols; engines need 32-aligned partition bases, hence the pad).
    Streaming w_hi then w_lo yields all four Veltkamp partial products,
    hi-terms in PSUM rows 0:16, lo-terms in rows 32:48; two DVE adds fold
    rows and x_proj into the gates.  16384 PE rows/step (the W-ingest
    floor for fp32-exact products) vs 24576 for 3-pass.
  * Gate column order is [i|f|o|g] host-side: one sigmoid over 384 cols +
    one tanh over 128 cols per block.
  * Hidden blocks are processed in slot order (3,0,1,2); all device-side
    chunk indexing (W layout, h.T state cols, c cols) is slot-major, so
    slots 0,1 batch their cell ops / splits with affine strided APs.
  * Per-block PE transposes of h are deferred into later matmul groups'
    PE-queue slots, and filler matmuls bridge the chain-latency stalls so
    the PE issue rate (1 row/cycle, ~233ns per 512-row matmul) never
    drops to a lower p-state.
  * The elementwise chain stays on DVE + Act (gpsimd/DMA have multi-us
    wake latencies that poison the recurrence-critical chain).
"""

import numpy as np

import concourse.bass as bass
import concourse.tile as tile
from concourse import bacc, mybir
from concourse.bass_utils import run_bass_kernel_spmd

B, F, H, TMAX = 128, 128, 512, 512
N_CORES = 8
BL = B // N_CORES          # local batch = 16
NB = 4                     # hidden blocks of 128
T_STEPS = TMAX - 1
SPLIT_C = float(2.0 ** 12 + 1)

SLOT2BLK = (3, 0, 1, 2)    # processing order: slot s handles hidden block SLOT2BLK[s]

FP32 = mybir.dt.float32
FP32R = mybir.dt.float32r
AF = mybir.ActivationFunctionType
OP = mybir.AluOpType


def _split12(x):
    x = x.astype(np.float32)
    v = (x * np.float32(SPLIT_C)).astype(np.float32)
    hi = (v - (v - x).astype(np.float32)).astype(np.float32)
    lo = (x - hi).astype(np.float32)
    return hi, lo


def build_lstm_nc(t_steps: int = T_STEPS):
    nc = bacc.Bacc("TRN2", target_bir_lowering=False, debug=False)

    # W_hh.T slot-major: wr*[:, ks*2048 + ns*512 + j] where ks/ns are SLOT
    # indices (hidden-in block SLOT2BLK[ks], gate-out block SLOT2BLK[ns]) and
    # j orders gates [i|f|o|g] x 128 within the block.
    wrh_d = nc.dram_tensor("wrh", [128, NB * 2048], FP32R, kind="ExternalInput")
    wrl_d = nc.dram_tensor("wrl", [128, NB * 2048], FP32R, kind="ExternalInput")
    wih_d = nc.dram_tensor("wih", [128, 2048], FP32R, kind="ExternalInput")
    wil_d = nc.dram_tensor("wil", [128, 2048], FP32R, kind="ExternalInput")
    z_d = nc.dram_tensor("z", [128, 2 * BL], FP32R, kind="ExternalInput")
    bias_d = nc.dram_tensor("bias", [BL, 2048], FP32, kind="ExternalInput")
    eye_d = nc.dram_tensor("eye", [128, 128], FP32, kind="ExternalInput")
    # ys slot-major on hid: ys[t, b, s*128+q] = h[b, SLOT2BLK[s]*128+q]
    ys_d = nc.dram_tensor("ys", [t_steps, BL, H], FP32, kind="ExternalOutput")

    with tile.TileContext(nc) as tc:
        with (
            tc.tile_pool(name="const", bufs=1) as constp,
            tc.tile_pool(name="state", bufs=1) as statep,
            tc.tile_pool(name="work", bufs=3) as workp,
            tc.tile_pool(name="hout", bufs=2) as houtp,
            tc.tile_pool(name="ps", bufs=3, space="PSUM") as psp,
            tc.tile_pool(name="pst", bufs=2, space="PSUM") as pstp,
            tc.tile_pool(name="fill", bufs=2, space="PSUM") as fillp,
            tc.tile_pool(name="xps", bufs=1, space="PSUM") as xpsp,
        ):
            wrh = constp.tile([128, NB * 2048], FP32R)
            nc.sync.dma_start(wrh[:], wrh_d.ap())
            wrl = constp.tile([128, NB * 2048], FP32R)
            nc.sync.dma_start(wrl[:], wrl_d.ap())
            wih = constp.tile([128, 2048], FP32R)
            nc.sync.dma_start(wih[:], wih_d.ap())
            wil = constp.tile([128, 2048], FP32R)
            nc.sync.dma_start(wil[:], wil_d.ap())
            z2 = constp.tile([128, 2 * BL], FP32R)
            nc.sync.dma_start(z2[:], z_d.ap())
            bias = constp.tile([BL, 2048], FP32)
            nc.sync.dma_start(bias[:BL, :], bias_d.ap())
            eye = constp.tile([128, 128], FP32)
            nc.sync.dma_start(eye[:], eye_d.ap())

            # --- x_proj once -> xp2 [32, 2048]: rows 0:16 xp, rows 16:32 zero
            xp2 = constp.tile([3 * BL, 2048], FP32)
            nc.vector.memset(xp2[: 3 * BL, :], 0.0)
            z_hi = z2[:, :BL]
            z_lo = z2[:, BL:]
            for s in range(NB):
                xps = xpsp.tile([BL, 512], FP32, tag="xps")
                wi_h = wih[:, s * 512 : (s + 1) * 512]
                wi_l = wil[:, s * 512 : (s + 1) * 512]
                nc.tensor.matmul(xps[:BL, :], z_hi, wi_h, start=True, stop=False)
                nc.tensor.matmul(xps[:BL, :], z_hi, wi_l, start=False, stop=False)
                nc.tensor.matmul(xps[:BL, :], z_lo, wi_h, start=False, stop=True)
                nc.vector.tensor_add(
                    xp2[:BL, s * 512 : (s + 1) * 512],
                    xps[:BL, :],
                    bias[:BL, s * 512 : (s + 1) * 512],
                )

            # --- state ---
            c1 = statep.tile([BL, H], FP32)          # slot-major cell state
            nc.vector.memset(c1[:BL, :], 0.0)
            # h.T hi/lo packed, slot-major: slot s hi at cols [32s,32s+16),
            # lo at [32s+16,32s+32).  Ping-pong.
            hT = [
                statep.tile([128, 192], FP32R, tag=f"hT{j}", name=f"hT{j}")
                for j in range(2)
            ]
            zf = statep.tile([128, 192], FP32)
            nc.vector.memset(zf[:], 0.0)
            nc.vector.tensor_copy(hT[0][:], zf[:])
            nc.vector.tensor_copy(hT[1][:], zf[:])

            # ---------------- recurrence ----------------
            # Deferred PE-op queue: list of (group_idx, ki, fn) executed when
            # the k-loop of that global group reaches position ki.
            sched: list = []

            def run_sched(g, ki):
                for item in list(sched):
                    if item[0] == g and item[1] == ki:
                        item[2]()
                        sched.remove(item)

            def mk_filler(n):
                def fn():
                    for _ in range(n):
                        psf = fillp.tile([BL, 512], FP32, tag="fill")
                        nc.tensor.matmul(psf[:BL, :], z_hi, wih[:, :512],
                                         start=True, stop=True)
                return fn

            def mk_transpose(pT, cols, hb_ap):
                def fn():
                    nc.tensor.transpose(pT[:, cols], hb_ap, eye[:BL, :BL])
                return fn

            def mk_split(pT, pcols, hn_t, scol, nblk):
                # nblk slots' worth: copy hi (fp32r rounds) then sub lo.
                def fn():
                    src = pT[:, pcols[0] : pcols[0] + 16 * nblk]
                    if nblk == 1:
                        hi = hn_t[:, scol : scol + 16]
                        lo = hn_t[:, scol + 32 : scol + 48]
                        nc.vector.tensor_copy(hi, src)
                        nc.vector.tensor_sub(lo, src, hi)
                    else:
                        hv = hn_t[:, :].rearrange("p (s c) -> p s c", c=48)
                        pv = src.rearrange("p (s c) -> p s c", c=16)
                        hi = hv[:, scol // 48 : scol // 48 + nblk, 0:16]
                        lo = hv[:, scol // 48 : scol // 48 + nblk, 32:48]
                        nc.vector.tensor_copy(hi, pv)
                        nc.vector.tensor_sub(lo, pv, hi)
                return fn

            for t in range(t_steps):
                hp = hT[t % 2]
                hn = hT[(t + 1) % 2]
                gbase = t * NB
                # per-step tiles
                ga2 = workp.tile([BL, 2048], FP32, tag="ga2")
                acts = workp.tile([BL, 2048], FP32, tag="acts")
                hbp = houtp.tile([BL, 256], FP32, tag="hbp")    # pair slots 0,1
                hb2 = houtp.tile([BL, 128], FP32, tag="hb2")    # slot 2
                hb3 = houtp.tile([BL, 128], FP32, tag="hb3")    # slot 3
                pT = pstp.tile([128, 64], FP32, tag="pT")       # 4 x [128,16]

                for s in range(NB):
                    g = gbase + s
                    ps = psp.tile([3 * BL, 512], FP32, tag="gates")
                    for ki in range(NB):
                        run_sched(g, ki)
                        w_h = wrh[:, ki * 2048 + s * 512 : ki * 2048 + (s + 1) * 512]
                        w_l = wrl[:, ki * 2048 + s * 512 : ki * 2048 + (s + 1) * 512]
                        st = hp[:, ki * 48 : ki * 48 + 3 * BL]
                        nc.tensor.matmul(ps[: 3 * BL, :], st, w_h,
                                         start=(ki == 0), stop=False)
                        nc.tensor.matmul(ps[: 3 * BL, :], st, w_l,
                                         start=False, stop=(ki == NB - 1))
                    # fold rows + xp in two 1-PSUM-operand steps:
                    #   slo = ps[32:48] + xp ; ga2 = ps[0:16] + slo
                    slo = workp.tile([BL, 512], FP32, tag="slo")
                    nc.vector.tensor_add(
                        slo[:BL, :],
                        ps[2 * BL : 3 * BL, :],
                        xp2[:BL, s * 512 : (s + 1) * 512],
                    )
                    nc.vector.tensor_add(
                        ga2[:BL, s * 512 : (s + 1) * 512],
                        ps[:BL, :], slo[:BL, :],
                    )

                    if s == 1:
                        # ---- batched pair: slots 0,1 ----
                        gv = ga2[:BL, :1024].rearrange("p (s c) -> p s c", c=512)
                        av = acts[:BL, :1024].rearrange("p (s c) -> p s c", c=512)
                        nc.scalar.activation(av[:, :, 0:384],
                                             gv[:, :, 0:384], AF.Sigmoid)
                        nc.scalar.activation(av[:, :, 384:512],
                                             gv[:, :, 384:512], AF.Tanh)
                        cv = c1[:BL, :256].rearrange("p (s c) -> p s c", c=128)
                        i_s = av[:, :, 0:128]
                        f_s = av[:, :, 128:256]
                        o_s = av[:, :, 256:384]
                        g_s = av[:, :, 384:512]
                        cn = cv[:, :, :]
                        t1 = workp.tile([BL, 256], FP32, tag="t1p")
                        t1v = t1[:BL, :].rearrange("p (s c) -> p s c", c=128)
                        nc.vector.tensor_mul(t1v, i_s, g_s)
                        nc.vector.tensor_mul(cn, f_s, cn)
                        nc.vector.tensor_add(cn, cn, t1v)
                        tct = workp.tile([BL, 256], FP32, tag="tctp")
                        tcv = tct[:BL, :].rearrange("p (s c) -> p s c", c=128)
                        nc.scalar.activation(tcv, cn, AF.Tanh)
                        hbv = hbp[:BL, :].rearrange("p (s c) -> p s c", c=128)
                        nc.vector.tensor_mul(hbv, o_s, tcv)
                        for ss in range(2):
                            nc.sync.dma_start(
                                ys_d.ap()[t, :, ss * 128 : (ss + 1) * 128],
                                hbp[:BL, ss * 128 : (ss + 1) * 128],
                            )
                        if t < t_steps - 1:
                            # pair transposes late in G3 (g = gbase+3)
                            sched.append((gbase + 3, 2, mk_filler(8)))
                            sched.append((gbase + 3, 2, mk_transpose(
                                pT, slice(0, 16), hbp[:BL, 0:128])))
                            sched.append((gbase + 3, 3, mk_transpose(
                                pT, slice(16, 32), hbp[:BL, 128:256])))
                            sched.append((gbase + 3, 3, mk_split(
                                pT, (0,), hn, 0, 2)))
                    elif s >= 2:
                        # ---- individual slot s ----
                        sl = slice(s * 512, (s + 1) * 512)
                        nc.scalar.activation(acts[:BL, s * 512 : s * 512 + 384],
                                             ga2[:BL, s * 512 : s * 512 + 384],
                                             AF.Sigmoid)
                        nc.scalar.activation(acts[:BL, s * 512 + 384 : (s + 1) * 512],
                                             ga2[:BL, s * 512 + 384 : (s + 1) * 512],
                                             AF.Tanh)
                        i_s = acts[:BL, s * 512 : s * 512 + 128]
                        f_s = acts[:BL, s * 512 + 128 : s * 512 + 256]
                        o_s = acts[:BL, s * 512 + 256 : s * 512 + 384]
                        g_s = acts[:BL, s * 512 + 384 : (s + 1) * 512]
                        cn = c1[:BL, s * 128 : (s + 1) * 128]
                        t1 = workp.tile([BL, 128], FP32, tag=f"t1_{s}")
                        nc.vector.tensor_mul(t1[:BL, :], i_s, g_s)
                        nc.vector.tensor_mul(cn, f_s, cn)
                        nc.vector.tensor_add(cn, cn, t1[:BL, :])
                        tct = workp.tile([BL, 128], FP32, tag=f"tct_{s}")
                        nc.scalar.activation(tct[:BL, :], cn, AF.Tanh)
                        hb = hb2 if s == 2 else hb3
                        nc.vector.tensor_mul(hb[:BL, :], o_s, tct[:BL, :])
                        nc.sync.dma_start(
                            ys_d.ap()[t, :, s * 128 : (s + 1) * 128], hb[:BL, :]
                        )
                        if t < t_steps - 1:
                            if s == 2:
                                sched.append((gbase + 4, 1, mk_filler(3)))
                                sched.append((gbase + 4, 1, mk_transpose(
                                    pT, slice(32, 48), hb[:BL, :])))
                                sched.append((gbase + 4, 1, mk_split(
                                    pT, (32,), hn, 96, 1)))
                            else:
                                sched.append((gbase + 4, 3, mk_filler(4)))
                                sched.append((gbase + 4, 3, mk_transpose(
                                    pT, slice(48, 64), hb[:BL, :])))
                                sched.append((gbase + 4, 3, mk_split(
                                    pT, (48,), hn, 144, 1)))
                # end slots
            # flush any leftover scheduled items (final step): drop them —
            # they only feed the next (nonexistent) step's state.
            sched.clear()

    nc.compile()
    return nc


def _prep_host_inputs(Z, seq_len, W_ih, W_hh, b_ih, b_hh):
    WT = np.ascontiguousarray(W_hh.astype(np.float32).T)      # [H, 4H]
    WIT = np.ascontiguousarray(W_ih.astype(np.float32).T)     # [F, 4H]
    bias = (b_ih.astype(np.float32) + b_hh.astype(np.float32))

    # column j within a block: gates ordered [i|f|o|g]; block ns is the
    # hidden block SLOT2BLK[ns].
    GORD = (0, 1, 3, 2)                                       # i,f,o,g
    j = np.arange(2048)
    ns, rem = np.divmod(j, 512)
    gj, qq = np.divmod(rem, 128)
    gate = np.array(GORD)[gj]
    blk = np.array(SLOT2BLK)[ns]
    colmap = gate * H + 128 * blk + qq                        # [2048]

    wr_np = np.empty((128, NB * 2048), dtype=np.float32)
    for ks in range(NB):
        kb = SLOT2BLK[ks]
        wr_np[:, ks * 2048 : (ks + 1) * 2048] = WT[kb * 128 : (kb + 1) * 128, colmap]
    wrh_np, wrl_np = _split12(wr_np)
    wih_np, wil_np = _split12(np.ascontiguousarray(WIT[:, colmap]))
    bias_np = np.broadcast_to(bias[colmap], (BL, 2048)).copy()
    eye_np = np.eye(128, dtype=np.float32)

    in_maps = []
    for c in range(N_CORES):
        zc = np.ascontiguousarray(Z[c * BL : (c + 1) * BL].astype(np.float32).T)
        z_hi, z_lo = _split12(zc)
        z_np = np.concatenate([z_hi, z_lo], axis=1)
        in_maps.append(
            {"wrh": wrh_np, "wrl": wrl_np, "wih": wih_np, "wil": wil_np,
             "z": z_np, "bias": bias_np, "eye": eye_np}
        )
    return in_maps


_NC_CACHE = {}


def get_nc(t_steps: int = T_STEPS):
    if t_steps not in _NC_CACHE:
        _NC_CACHE[t_steps] = build_lstm_nc(t_steps)
    return _NC_CACHE[t_steps]


def kernel(Z, seq_len, W_ih, W_hh, b_ih, b_hh, _trace=False, _tmpdir=None):
    nc = get_nc()
    in_maps = _prep_host_inputs(Z, seq_len, W_ih, W_hh, b_ih, b_hh)
    res = run_bass_kernel_spmd(
        nc, in_maps, core_ids=list(range(N_CORES)), trace=_trace, tmpdir=_tmpdir
    )
    kernel.last_result = res

    out = np.zeros((B, TMAX, H), dtype=np.float32)
    # undo slot-major hid ordering: device col s*128+q -> hid SLOT2BLK[s]*128+q
    perm = np.concatenate([np.arange(128) + 128 * b for b in SLOT2BLK])
    inv = np.empty(H, dtype=np.int64)
    inv[perm] = np.arange(H)
    for c in range(N_CORES):
        ys = res.results[c]["ys"]  # [T_STEPS, BL, H] slot-major
        out[c * BL : (c + 1) * BL, :T_STEPS] = ys[:, :, inv].transpose(1, 0, 2)
    mask = np.arange(TMAX, dtype=np.int64)[None, :] < seq_len.astype(np.int64)[:, None]
    out *= mask[:, :, None].astype(np.float32)
    return out
